# revision 15
# baseline (speedup 1.0000x reference)
"""Trainium2 Bass kernel for nn_CLOSEgaps (hypergraph attention conv), 8 NeuronCores.

Algorithmic collapse (validated vs reference):
  nodes16 = node_idx.reshape(M,16); per-node 12-vector table:
    s_n[h] = x @ (W_conv_h @ att[h,:128]);  p[h,c] = x @ (W_conv_h @ W_out)
    wav[h] = (W_attr + b_attr/16) @ (W_conv_h @ att[h,128:])
  per-pair: e = exp(lrelu(s_n + s_e)), Z, T, qq, v (2 f32)  — see baseline.
  zacc[n,c] = sum of v over pairs at n;  z = Dv*zacc + C
  out[m,c] = sum_{n in e_m} z[n,c]  (fp8 incidence matmul on PE)

v2 changes vs baseline (601 us -> target ~250 us):
  - Gather descriptors are generated EARLY (prepare_only=True) on GpSimd,
    overlapping the encoder + table AllGather; one trigger_dma fires them
    once the table lands. (desc-gen at ~8 ns/idx was the serial bottleneck.)
  - The HBM dma_scatter_add + lane table + lane reduce (200+ us) is replaced
    by an in-SBUF permutation network: v (16384 pairs x 2 f32, bf16 planes)
    is routed to node-sorted (node, lane<16) slots via
    R1 -> T -> R2 -> T -> R3 (local_scatter ucode for R stages, PE
    transposes for T), then a vector lane-reduce gives zacc [8192, 2]
    per-core partials, AllReduced directly (64 KiB).
  - Input DMA is priority-ordered; the 8 MiB fp8 incidence loads last.
"""
import sys

sys.path.insert(0, "/opt/trn_rl_repo")

import numpy as np

N = 8192
M = 8192
F_IN = 512
EMB = 256
CD = 128
H = 3
DEG = 16
NCORES = 8
NL = N // NCORES      # nodes per core
ML = M // NCORES      # edges per core
EL = ML * DEG         # pairs per core = 16384
GCH = 2048            # gather chunk (indices per ucode call)
ELEM = 64             # f32 per gather-table row (ucode needs 256B rows)
LANES = 16            # node-slot lanes in the permuted layout

_CACHE = {}
import os
PREP_MODE = int(os.environ.get("KPREP", "1"))  # 0: direct gathers, 1: prepare_only+trigger


def _build_program():
    import concourse.bass as bass
    import concourse.bacc as bacc
    import concourse.tile as tile
    from concourse import mybir

    f32 = mybir.dt.float32
    i16 = mybir.dt.int16
    bf16 = mybir.dt.bfloat16

    nc = bacc.Bacc("TRN2", target_bir_lowering=False, debug=False, num_devices=NCORES,
                   num_swdge_queues=4 if PREP_MODE else 1,
                   dynamic_dma_scratch_size=32768 if PREP_MODE else 16384)

    # ---- per-core external inputs (host pre-laid-out) ----
    idxg_in = nc.dram_tensor("idxg_in", [128, EL // 16], i16, kind="ExternalInput").ap()
    ift_in = nc.dram_tensor("ift_in", [128, 4 * 1024], f32, kind="ExternalInput").ap()
    wenc_in = nc.dram_tensor("wenc_in", [128, 4 * 256], f32, kind="ExternalInput").ap()
    benc_in = nc.dram_tensor("benc_in", [128, 2], f32, kind="ExternalInput").ap()
    wattr_in = nc.dram_tensor("wattr_in", [128, 2 * 1024], f32, kind="ExternalInput").ap()
    wconv_in = nc.dram_tensor("wconv_in", [128, 3 * 256], f32, kind="ExternalInput").ap()
    pproj_in = nc.dram_tensor("pproj_in", [128, 3 * 12], f32, kind="ExternalInput").ap()
    r1_in = nc.dram_tensor("r1_in", [128, 128], i16, kind="ExternalInput").ap()
    r2_in = nc.dram_tensor("r2_in", [128, 128], i16, kind="ExternalInput").ap()
    r3_in = nc.dram_tensor("r3_in", [128, 1024], i16, kind="ExternalInput").ap()
    dv_in = nc.dram_tensor("dv_in", [128, 64], f32, kind="ExternalInput").ap()
    cc_in = nc.dram_tensor("cc_in", [128, 2], f32, kind="ExternalInput").ap()
    inc_in = nc.dram_tensor("inc_in", [128, 8 * 64 * 128], mybir.dt.float8e4, kind="ExternalInput").ap()
    ident_in = nc.dram_tensor("ident_in", [128, 128], mybir.dt.bfloat16, kind="ExternalInput").ap()
    out_dram = nc.dram_tensor("out", [128, 16], f32, kind="ExternalOutput").ap()

    with tile.TileContext(nc) as tc:
        with (
            tc.tile_pool(name="sbuf", bufs=1) as sb,
            tc.tile_pool(name="big", bufs=1) as bigp,
            tc.tile_pool(name="psum", bufs=2, space="PSUM") as ps,
            tc.tile_pool(name="psum1", bufs=1, space="PSUM") as ps1,
            tc.tile_pool(name="dram", bufs=1, space="DRAM") as dram,
        ):
            # ------- critical input loads (priority order) -------
            idxg_t = sb.tile([128, EL // 16], i16)
            nc.sync.dma_start(out=idxg_t[:], in_=idxg_in[:])
            wenc_t = sb.tile([128, 4, 256], f32)
            nc.sync.dma_start(out=wenc_t[:].rearrange("p a b -> p (a b)"), in_=wenc_in[:])
            benc_t = sb.tile([128, 2], f32)
            nc.sync.dma_start(out=benc_t[:], in_=benc_in[:])
            ift_t = sb.tile([128, 4, 1024], f32)
            nc.sync.dma_start(out=ift_t[:].rearrange("p a b -> p (a b)"), in_=ift_in[:])
            wattr_t = sb.tile([128, 2, 1024], f32)
            nc.sync.dma_start(out=wattr_t[:].rearrange("p a b -> p (a b)"), in_=wattr_in[:])
            wconv_t = sb.tile([128, 3, 256], f32)
            nc.sync.dma_start(out=wconv_t[:].rearrange("p a b -> p (a b)"), in_=wconv_in[:])
            pproj_t = sb.tile([128, 3, 12], f32)
            nc.sync.dma_start(out=pproj_t[:].rearrange("p a b -> p (a b)"), in_=pproj_in[:])
            r1_t = sb.tile([128, 128], i16)
            nc.sync.dma_start(out=r1_t[:], in_=r1_in[:])
            r2_t = sb.tile([128, 128], i16)
            nc.sync.dma_start(out=r2_t[:], in_=r2_in[:])
            r3_t = sb.tile([128, 1024], i16)
            nc.sync.dma_start(out=r3_t[:], in_=r3_in[:])
            dv_t = sb.tile([128, 64], f32)
            nc.sync.dma_start(out=dv_t[:], in_=dv_in[:])
            cc_t = sb.tile([128, 2], f32)
            nc.sync.dma_start(out=cc_t[:], in_=cc_in[:])

            table_full = dram.tile([N, ELEM], f32)
            G = bigp.tile([128, 128, ELEM], f32, tag="gz")
            dma_sems = [nc.alloc_semaphore(f"gdma{g}") for g in range(EL // GCH)]
            fence_sems = [nc.alloc_semaphore(f"fence{q}") for q in range(4)]

            # ------- gather desc-gen (prepare_only), split around AG trigger --
            def prep(g):
                if PREP_MODE:
                    nc.gpsimd.dma_gather(
                        out_ap=G[:, g * (GCH // 128) : (g + 1) * (GCH // 128), :],
                        in_ap=table_full[:],
                        idxs_ap=idxg_t[:, g * (GCH // 16) : (g + 1) * (GCH // 16)],
                        num_idxs=GCH,
                        num_idxs_reg=GCH,
                        elem_size=ELEM,
                        single_packet=False,
                        prepare_only=True,
                        sem=dma_sems[g],
                        queue_num=g % 4,
                    )

            def gather_direct(g):
                nc.gpsimd.dma_gather(
                    out_ap=G[:, g * (GCH // 128) : (g + 1) * (GCH // 128), :],
                    in_ap=table_full[:],
                    idxs_ap=idxg_t[:, g * (GCH // 16) : (g + 1) * (GCH // 16)],
                    num_idxs=GCH,
                    num_idxs_reg=GCH,
                    elem_size=ELEM,
                    single_packet=False,
                )

            prep(0)
            prep(1)

            # ------- P1: xT[e, n] = relu(W_enc.T @ IF.T + b_enc), emb-major --
            xT_t = sb.tile([128, 2, 1024], f32)
            for eh in range(2):
                for nh in range(2):
                    px = ps.tile([128, 512], f32, tag="px")
                    for kc in range(4):
                        nc.tensor.matmul(
                            out=px[:],
                            lhsT=wenc_t[:, kc, eh * 128 : (eh + 1) * 128],
                            rhs=ift_t[:, kc, nh * 512 : (nh + 1) * 512],
                            start=(kc == 0),
                            stop=(kc == 3),
                        )
                    nc.scalar.activation(
                        out=xT_t[:, eh, nh * 512 : (nh + 1) * 512],
                        in_=px[:],
                        func=mybir.ActivationFunctionType.Relu,
                        bias=benc_t[:, eh : eh + 1],
                    )

            # ------- P2: UV = W_conv.T-chunks @ P_proj  ([256(2x128), 12]) --
            uv_t = sb.tile([128, 2, 12], f32)
            for eh in range(2):
                pu = ps.tile([128, 12], f32, tag="pu")
                for qc in range(3):
                    nc.tensor.matmul(
                        out=pu[:],
                        lhsT=wconv_t[:, qc, eh * 128 : (eh + 1) * 128],
                        rhs=pproj_t[:, qc, :],
                        start=(qc == 0),
                        stop=(qc == 2),
                    )
                nc.vector.tensor_copy(uv_t[:, eh, :], pu[:])

            # ------- P2b: node-major table rows: staging[p, nb, 0:12] -------
            staging = sb.tile([128, 8, 12], f32)
            for nb in range(8):
                pn = ps.tile([128, 12], f32, tag="pn")
                for eh in range(2):
                    nc.tensor.matmul(
                        out=pn[:, :9],
                        lhsT=xT_t[:, eh, nb * 128 : (nb + 1) * 128],
                        rhs=uv_t[:, eh, 0:9],
                        start=(eh == 0),
                        stop=(eh == 1),
                    )
                for ec in range(2):
                    nc.tensor.matmul(
                        out=pn[:, 9:12],
                        lhsT=wattr_t[:, ec, nb * 128 : (nb + 1) * 128],
                        rhs=uv_t[:, ec, 9:12],
                        start=(ec == 0),
                        stop=(ec == 1),
                    )
                nc.vector.tensor_copy(staging[:, nb, :], pn[:])

            gatetmp = sb.tile([128, 2], f32)
            nc.vector.tensor_copy(gatetmp[:], staging[:, 7, 0:2])

            # ------- AllGather compact node table, then expand to 256B rows --
            tslice = dram.tile([NL, 12], f32)
            nc.sync.dma_start(
                out=tslice[:].rearrange("(p nb) e -> p nb e", p=128), in_=staging[:]
            )
            table_c = dram.tile([N, 12], f32)
            nc.gpsimd.collective_compute(
                "AllGather",
                mybir.AluOpType.bypass,
                replica_groups=[list(range(NCORES))],
                ins=[tslice.opt()],
                outs=[table_c.opt()],
            )
            TS = sb.tile([128, 64, 12], f32)
            nc.sync.dma_start(
                out=TS[:], in_=table_c[:].rearrange("(a p) e -> p a e", p=128)
            )
            nc.sync.dma_start(
                out=table_full[:].rearrange("(a p) e -> p a e", p=128)[:, :, 0:12],
                in_=TS[:],
            )

            # ------- remaining gather desc-gens, then fire the ring -------
            if PREP_MODE:
                for g in range(2, EL // GCH):
                    prep(g)
                for qn in range(4):
                    nc.gpsimd.trigger_dma(count=None, queue_num=qn)
                # per-queue fences: normal gathers whose DMA sems fire at true
                # completion; ring FIFO per queue orders them after the preps
                scrap = bigp.tile([128, 4, ELEM], f32)
                for qn in range(4):
                    nc.gpsimd.dma_gather(
                        out_ap=scrap[:, qn : qn + 1, :], in_ap=table_full[:],
                        idxs_ap=idxg_t[:, 0:8], num_idxs=128, num_idxs_reg=128,
                        elem_size=ELEM, single_packet=False, queue_num=qn,
                    ).then_inc(fence_sems[qn], 16)
            else:
                for g in range(EL // GCH):
                    gather_direct(g)

            # ------- non-critical loads (queue behind the critical ones) ----
            incs_t = bigp.tile([128, 8, 64, 128], mybir.dt.float8e4)
            nc.sync.dma_start(out=incs_t[:].rearrange("p a b m -> p (a b m)"), in_=inc_in[:])

            ident = sb.tile([128, 128], bf16)
            nc.sync.dma_start(out=ident[:], in_=ident_in[:])

            # ------- attention per edge-block mb; v -> vt[p, blk, c] -------
            vt = sb.tile([128, 128, 2], f32)
            for mb in range(8):
                blk = slice(16 * mb, 16 * mb + 16)
                if PREP_MODE:
                    ins_gate = nc.vector.scalar_tensor_tensor(
                        out=G[:, blk, 0:12],
                        in0=gatetmp[:, None, 0:1].to_broadcast([128, 16, 12]),
                        scalar=0.0,
                        in1=G[:, blk, 0:12],
                        op0=mybir.AluOpType.mult,
                        op1=mybir.AluOpType.add,
                    )
                    ins_gate._wait_ge(fence_sems[mb % 4], 16)
                g_sn = G[:, blk, 0:3]                      # [p, k, h]
                g_pp = G[:, blk, 3:9]                      # [p, k, 6]
                g_wv = G[:, blk, 9:12]                     # [p, k, h]
                se = sb.tile([128, 3], f32, tag="se")
                ins_se = nc.vector.reduce_sum(
                    out=se[:][:, :, None],
                    in_=g_wv.rearrange("p k h -> p h k"),
                    axis=mybir.AxisListType.X,
                )

                lg = sb.tile([128, 16, 3], f32, tag="lg")
                ins_lg = nc.vector.tensor_tensor(
                    out=lg[:],
                    in0=g_sn,
                    in1=se[:][:, None, :].to_broadcast([128, 16, 3]),
                    op=mybir.AluOpType.add,
                )

                lgs = sb.tile([128, 16, 3], f32, tag="lgs")
                nc.vector.tensor_scalar_mul(lgs[:], lg[:], 0.2)
                nc.vector.tensor_tensor(
                    out=lg[:], in0=lg[:], in1=lgs[:], op=mybir.AluOpType.max
                )
                ee = sb.tile([128, 16, 3], f32, tag="ee")
                nc.scalar.activation(
                    out=ee[:], in_=lg[:], func=mybir.ActivationFunctionType.Exp
                )
                zz = sb.tile([128, 3], f32, tag="zz")
                nc.vector.reduce_sum(
                    out=zz[:][:, :, None],
                    in_=ee[:].rearrange("p k h -> p h k"),
                    axis=mybir.AxisListType.X,
                )
                nc.vector.tensor_scalar_add(zz[:], zz[:], 1e-16)
                zr = sb.tile([128, 3], f32, tag="zr")
                nc.vector.reciprocal(zr[:], zz[:])
                tq = sb.tile([128, 16, 6], f32, tag="tq")
                ins_tq = nc.vector.tensor_tensor(
                    out=tq[:].rearrange("p k (h c) -> p k h c", c=2),
                    in0=g_pp.rearrange("p k (h c) -> p k h c", c=2),
                    in1=ee[:][:, :, :, None].to_broadcast([128, 16, 3, 2]),
                    op=mybir.AluOpType.mult,
                )

                tt = sb.tile([128, 6], f32, tag="tt")
                nc.vector.reduce_sum(
                    out=tt[:].rearrange("p (h c) -> p h c", c=2)[:, :, :, None],
                    in_=tq[:].rearrange("p k (h c) -> p h c k", c=2),
                    axis=mybir.AxisListType.X,
                )
                zr2 = sb.tile([128, 3], f32, tag="zr2")
                nc.vector.tensor_tensor(
                    out=zr2[:], in0=zr[:], in1=zr[:], op=mybir.AluOpType.mult
                )
                nc.vector.tensor_scalar_mul(zr2[:], zr2[:], 1.0 / DEG)
                qq = sb.tile([128, 3, 2], f32, tag="qq")
                nc.vector.tensor_tensor(
                    out=qq[:],
                    in0=tt[:].rearrange("p (h c) -> p h c", c=2),
                    in1=zr2[:][:, :, None].to_broadcast([128, 3, 2]),
                    op=mybir.AluOpType.mult,
                )
                vh = sb.tile([128, 16, 3, 2], f32, tag="vh")
                nc.vector.tensor_tensor(
                    out=vh[:],
                    in0=ee[:][:, :, :, None].to_broadcast([128, 16, 3, 2]),
                    in1=qq[:][:, None, :, :].to_broadcast([128, 16, 3, 2]),
                    op=mybir.AluOpType.mult,
                )
                nc.vector.reduce_sum(
                    out=vt[:, blk, :][:, :, :, None],
                    in_=vh[:].rearrange("p k h c -> p k c h"),
                    axis=mybir.AxisListType.X,
                )

            # ------- v comp planes (bf16) -------
            V0 = sb.tile([128, 128], bf16)
            V1 = sb.tile([128, 128], bf16)
            nc.vector.tensor_copy(V0[:], vt[:, :, 0])
            nc.vector.tensor_copy(V1[:], vt[:, :, 1])

            # ------- permutation network: R1 -> T -> R2 -> T -> R3 -------
            S1a = sb.tile([128, 128], bf16)
            S1b = sb.tile([128, 128], bf16)
            S1Ta = sb.tile([128, 128], bf16)
            S1Tb = sb.tile([128, 128], bf16)
            S2a = sb.tile([128, 1024], bf16)
            S2b = sb.tile([128, 1024], bf16)
            S2Ta = sb.tile([128, 1024], bf16)
            S2Tb = sb.tile([128, 1024], bf16)
            S3a = sb.tile([128, 1024], bf16)
            S3b = sb.tile([128, 1024], bf16)
            S1 = [S1a, S1b]
            S1T = [S1Ta, S1Tb]
            S2 = [S2a, S2b]
            S2T = [S2Ta, S2Tb]
            S3 = [S3a, S3b]
            for c, V in enumerate([V0, V1]):
                nc.gpsimd.local_scatter(
                    out_ap=S1[c][:], data_ap=V[:], idxs_ap=r1_t[:],
                    channels=128, num_elems=128, num_idxs=128,
                )
            for c in range(2):
                pt = ps1.tile([128, 128], bf16, tag="pt")
                nc.tensor.transpose(pt[:], S1[c][:], ident[:])
                nc.vector.tensor_copy(S1T[c][:], pt[:])
            for c in range(2):
                nc.gpsimd.local_scatter(
                    out_ap=S2[c][:], data_ap=S1T[c][:], idxs_ap=r2_t[:],
                    channels=128, num_elems=1024, num_idxs=128,
                )
            for c in range(2):
                for sq in range(8):
                    pt = ps1.tile([128, 128], bf16, tag="pt")
                    nc.tensor.transpose(
                        pt[:], S2[c][:, 128 * sq : 128 * (sq + 1)], ident[:]
                    )
                    nc.vector.tensor_copy(S2T[c][:, 128 * sq : 128 * (sq + 1)], pt[:])
            for c in range(2):
                nc.gpsimd.local_scatter(
                    out_ap=S3[c][:], data_ap=S2T[c][:], idxs_ap=r3_t[:],
                    channels=128, num_elems=1024, num_idxs=1024,
                )

            # ------- lane reduce: zacc[p, nr, c], node = nr*128 + p -------
            zacc = sb.tile([128, 64, 2], f32)
            for c in range(2):
                nc.vector.reduce_sum(
                    out=zacc[:, :, c][:, :, None],
                    in_=S3[c][:].rearrange("p (nr l) -> p nr l", l=LANES),
                    axis=mybir.AxisListType.X,
                )

            # ------- AllReduce zacc -------
            ar_in = dram.tile([128, 128], f32)
            nc.sync.dma_start(out=ar_in[:], in_=zacc[:].rearrange("p a b -> p (a b)"))
            ar_out = dram.tile([128, 128], f32)
            nc.gpsimd.collective_compute(
                "AllReduce",
                mybir.AluOpType.add,
                replica_groups=[list(range(NCORES))],
                ins=[ar_in.opt()],
                outs=[ar_out.opt()],
            )
            zred = sb.tile([128, 64, 2], f32)
            nc.sync.dma_start(out=zred[:].rearrange("p a b -> p (a b)"), in_=ar_out[:])

            # ------- z = Dv*zacc + C; bf16 hi/lo split -------
            nc.vector.tensor_tensor(
                out=zred[:],
                in0=zred[:],
                in1=dv_t[:][:, :, None].to_broadcast([128, 64, 2]),
                op=mybir.AluOpType.mult,
            )
            nc.vector.tensor_tensor(
                out=zred[:],
                in0=zred[:],
                in1=cc_t[:][:, None, :].to_broadcast([128, 64, 2]),
                op=mybir.AluOpType.add,
            )
            zz4 = sb.tile([128, 64, 4], bf16)
            nc.vector.tensor_copy(zz4[:, :, 0:2], zred[:])
            zhi32 = sb.tile([128, 64, 2], f32)
            nc.vector.tensor_copy(zhi32[:], zz4[:, :, 0:2])
            nc.vector.tensor_tensor(
                out=zhi32[:], in0=zred[:], in1=zhi32[:], op=mybir.AluOpType.subtract
            )
            nc.vector.tensor_copy(zz4[:, :, 2:4], zhi32[:])

            # ------- final: out[p_e, c] = sum_n inc[n, edge] * z[n] via PE --
            out_t = sb.tile([128, 8, 2], f32)
            for j in range(8):
                po = ps1.tile([128, 4], f32, tag="po")
                for nck in range(64):
                    nc.tensor.matmul(
                        out=po[:],
                        lhsT=incs_t[:, j, nck, :],
                        rhs=zz4[:, nck, :],
                        start=(nck == 0),
                        stop=(nck == 63),
                    )
                nc.vector.tensor_copy(out_t[:, j, :], po[:, 0:2])
                nc.vector.tensor_tensor(
                    out=out_t[:, j, :], in0=out_t[:, j, :], in1=po[:, 2:4],
                    op=mybir.AluOpType.add,
                )
            nc.sync.dma_start(
                out=out_dram[:], in_=out_t[:].rearrange("p a b -> p (a b)")
            )

    nc.compile()
    return nc


def _host_prep(inputs):
    """Build per-core in_maps from full inputs."""
    IF = np.asarray(inputs["input_features"], np.float32)
    node_idx = np.asarray(inputs["node_idx"])
    W_enc = np.asarray(inputs["W_enc"], np.float32)
    b_enc = np.asarray(inputs["b_enc"], np.float32)
    W_attr = np.asarray(inputs["W_attr"], np.float32)
    b_attr = np.asarray(inputs["b_attr"], np.float32)
    W_conv = np.asarray(inputs["W_conv"], np.float32)
    att = np.asarray(inputs["att"], np.float32)
    b_conv = np.asarray(inputs["b_conv"], np.float32)
    W_out = np.asarray(inputs["W_out"], np.float32)
    b_out = np.asarray(inputs["b_out"], np.float32)

    nodes16 = node_idx.reshape(M, DEG).astype(np.int64)

    # weight prep
    P_proj = np.zeros((H * CD, 12), np.float32)
    for h in range(H):
        P_proj[h * CD : (h + 1) * CD, h] = att[h, :CD]
        for cc in range(2):
            P_proj[h * CD : (h + 1) * CD, 3 + h * 2 + cc] = W_out[h * CD : (h + 1) * CD, cc]
        P_proj[h * CD : (h + 1) * CD, 9 + h] = att[h, CD:]

    deg_n = np.bincount(node_idx, minlength=N)
    Dv = np.where(deg_n > 0, 1.0 / np.maximum(deg_n, 1), 0.0).astype(np.float32)
    C = (b_conv @ W_out + b_out / DEG).astype(np.float32)

    wenc_l = W_enc.reshape(4, 128, EMB).transpose(1, 0, 2).reshape(128, -1).copy()
    benc_l = b_enc.reshape(2, 128).T.copy()
    wconv_l = W_conv.T.reshape(3, 128, EMB).transpose(1, 0, 2).reshape(128, -1).copy()
    pproj_l = P_proj.reshape(3, 128, 12).transpose(1, 0, 2).reshape(128, -1).copy()
    cc_l = np.tile(C[None, :], (128, 1)).copy()
    dv_l = Dv.reshape(64, 128).T.copy()

    # global table row ids: node (c', nl) -> row 1024*c' + (nl%128)*8 + nl//128
    tab_row = (nodes16 // NL) * NL + (nodes16 % NL) % 128 * 8 + (nodes16 % NL) // 128

    def wrap16(a):
        return np.tile(a.reshape(-1, 16).T, (8, 1)).astype(np.int16).copy()

    # pair order: i = (jj*16+k)*128 + p ; local edge m = jj*128 + p
    i = np.arange(EL)
    p_of = i % 128
    blk = i // 128          # = f1 source free position
    jj = blk // 16
    k_of = blk % 16
    m_loc = jj * 128 + p_of

    import ml_dtypes

    ident_np = np.eye(128, dtype=ml_dtypes.bfloat16)
    in_maps = []
    for c in range(NCORES):
        nsl = slice(c * NL, (c + 1) * NL)
        esl = slice(c * ML, (c + 1) * ML)
        ift_l = (
            IF[nsl].T.reshape(4, 128, 1024).transpose(1, 0, 2).reshape(128, -1).copy()
        )
        wattr_l = (
            (W_attr[nsl] + b_attr[None, :] / DEG)
            .T.reshape(2, 128, 1024)
            .transpose(1, 0, 2)
            .reshape(128, -1)
            .copy()
        )
        nsub = nodes16[esl]                       # [1024, 16]
        tsub = tab_row[esl]
        idx_flat = tsub[m_loc, k_of]
        node_of = nsub[m_loc, k_of]               # node id per pair i

        # ---- permutation routing ----
        rng = np.random.default_rng(1234 + c)
        p2 = (node_of % 128).astype(np.int64)
        for _try in range(50):
            perms = np.stack([rng.permutation(128) for _ in range(128)])
            q = perms[p_of, blk]
            loads = np.zeros((128, 128), np.int64)
            np.add.at(loads, (q, p2), 1)
            if loads.max() <= 8:
                break
        else:
            raise RuntimeError("routing failed")

        def ranks_of(key):
            order = np.argsort(key, kind="stable")
            sk = key[order]
            seg_start = np.r_[0, np.nonzero(sk[1:] != sk[:-1])[0] + 1]
            starts = np.repeat(seg_start, np.diff(np.r_[seg_start, len(sk)]))
            r = np.empty(len(sk), np.int64)
            r[order] = np.arange(len(sk)) - starts
            return r

        sq2 = ranks_of(q * 128 + p2)
        assert sq2.max() < 8
        lane = ranks_of(node_of)
        assert lane.max() < LANES, f"lane overflow {lane.max()}"

        r1 = perms.astype(np.int16)               # r1[p1, f1] = q
        r2 = np.empty((128, 128), np.int16)
        r2[q, p_of] = (sq2 * 128 + p2).astype(np.int16)
        r3 = np.full((128, 1024), -1, np.int16)
        r3[p2, sq2 * 128 + q] = ((node_of // 128) * LANES + lane).astype(np.int16)

        # ---- fp8 incidence for the final matmul (identity edge grouping) --
        inc8 = np.zeros((N, 8, 128), np.float32)   # [node, j, m-col]
        mm = np.arange(ML)
        inc8[nsub, (mm // 128)[:, None], (mm % 128)[:, None]] = 1.0
        inc8 = inc8.reshape(64, 128, 8, 128).transpose(1, 2, 0, 3)
        inc8 = inc8.astype(ml_dtypes.float8_e4m3).reshape(128, -1).copy()

        in_maps.append(
            {
                "idxg_in": wrap16(idx_flat),
                "ift_in": ift_l,
                "wenc_in": wenc_l,
                "benc_in": benc_l,
                "wattr_in": wattr_l,
                "wconv_in": wconv_l,
                "pproj_in": pproj_l,
                "r1_in": r1,
                "r2_in": r2,
                "r3_in": r3,
                "dv_in": dv_l,
                "cc_in": cc_l,
                "inc_in": inc8,
                "ident_in": ident_np,
            }
        )
    return in_maps


LAST_RESULT = None


def kernel(**inputs):
    global LAST_RESULT
    from concourse import bass_utils

    if "nc" not in _CACHE:
        _CACHE["nc"] = _build_program()
    nc = _CACHE["nc"]
    in_maps = _host_prep(inputs)
    res = bass_utils.run_bass_kernel_spmd(
        nc, in_maps, core_ids=list(range(NCORES))
    )
    LAST_RESULT = res
    out = np.empty((M, 2), np.float32)
    for c in range(NCORES):
        o = res.results[c]["out"].reshape(128, 8, 2)   # [p, j, c]
        out[c * ML : (c + 1) * ML] = o.transpose(1, 0, 2).reshape(ML, 2)
    return out
